# revision 16
# baseline (speedup 1.0000x reference)
"""Trainium2 Bass kernel for nn_AutoeclecticResponderHead.

Math (per row b):
    w      = softmax(se_b * gate_w + gate_b)          # [4]
    mix    = sigmoid(curv_b)
    out_b  = (1-mix) * (state_b @ prj_w + prj_b) + mix * sum_m w_m * (state_b @ W_m)
           = sum_{k=0..4} c_k[b] * (state_b @ A_k)  +  c_4[b] * prj_b
    with A_0..3 = modulation_basis modes (c_k = mix*w_k), A_4 = prj_w (c_4 = 1-mix).

Sharding: data-parallel over batch, 1024 rows per core, weights replicated.
Per-core kernel: DMA fp32, cast to bf16 on device (ScalarE/VectorE), 640 bf16
matmuls ([128,128] stationary state-tile x [128,512] moving weight-tile)
accumulating 8 h-tiles per PSUM bank, then a fused scalar_tensor_tensor
(acc += c_k * psum) combine on the vector engine.
"""

import os
import numpy as np

B, H, O, M = 8192, 1024, 1024, 4
NCORES = 8
BL = B // NCORES          # rows per core
NB = BL // 128            # b tiles per core
NH = H // 128             # h (contraction) tiles
NO = O // 512             # output column tiles

_cached_nc = None
LAST_EXEC_TIME_NS = None
LAST_TRACE = None


def _build_nc():
    import concourse.bacc as bacc
    import concourse.tile as tile
    from concourse import mybir

    f32 = mybir.dt.float32
    bf16 = mybir.dt.bfloat16
    Alu = mybir.AluOpType
    Act = mybir.ActivationFunctionType
    AxX = mybir.AxisListType.X

    nc = bacc.Bacc("TRN2", target_bir_lowering=False, debug=False,
                   num_devices=NCORES)

    stateT = nc.dram_tensor("stateT", [H, BL], f32, kind="ExternalInput").ap()
    sc = nc.dram_tensor("sc", [128, 2 * NB], f32, kind="ExternalInput").ap()
    basis = nc.dram_tensor("basis", [M, H, O], f32, kind="ExternalInput").ap()
    prj_w = nc.dram_tensor("prj_w", [H, O], f32, kind="ExternalInput").ap()
    gwb = nc.dram_tensor("gwb", [128, 2 * M], f32, kind="ExternalInput").ap()
    pb = nc.dram_tensor("pb", [128, O], f32, kind="ExternalInput").ap()
    out = nc.dram_tensor("out", [BL, O], f32, kind="ExternalOutput").ap()

    st_r = stateT.rearrange("(t p) b -> p t b", p=128)          # [128, NH, BL]
    out_r = out.rearrange("(t p) o -> p t o", p=128)            # [128, NB, O]
    w_srcs = [basis[k].rearrange("(t p) o -> p t o", p=128) for k in range(M)]
    w_srcs.append(prj_w.rearrange("(t p) o -> p t o", p=128))

    with tile.TileContext(nc) as tc:
        with (
            tc.tile_pool(name="big", bufs=1) as bigpool,
            tc.tile_pool(name="stf", bufs=NB) as stfpool,
            tc.tile_pool(name="w", bufs=2 * NH) as wpool,
            tc.tile_pool(name="wb", bufs=3 * NH) as wbpool,
            tc.tile_pool(name="acc", bufs=NB) as apool,
            tc.tile_pool(name="g", bufs=NB) as gpool,
            tc.tile_pool(name="c", bufs=NB) as cpool,
            tc.tile_pool(name="ps", bufs=8, space="PSUM") as ppool,
        ):
            # Weight chunk (o,k) = 8 h-pieces, each its own tile so each
            # matmul depends only on its own piece's DMA+cast chain.
            def load_w_chunk(o, k):
                osl = slice(o * 512, (o + 1) * 512)
                pieces = []
                for h in range(NH):
                    wf = wpool.tile([128, 512], f32, tag="w")
                    nc.sync.dma_start(wf[:], w_srcs[k][:, h, osl])
                    wb = wbpool.tile([128, 512], bf16, tag="wb")
                    nc.scalar.copy(wb[:], wf[:])
                    pieces.append(wb)
                return pieces

            # First weight chunk goes first on the Sync DMA queue (its
            # pieces gate the first matmuls), then 8 contiguous per-h
            # stateT row DMAs (fast issue + transfer). Each h gets its own
            # staging tile and bf16 tile so every matmul depends only on
            # its own h's DMA+cast chain.
            wchunk = load_w_chunk(0, 0)
            stbh = []
            for h in range(NH):
                stf = stfpool.tile([128, BL], f32, tag="stf")
                nc.sync.dma_start(stf[:], st_r[:, h, :])
                sb = bigpool.tile([128, BL], bf16, tag=f"stbh{h}")
                nc.vector.tensor_copy(sb[:], stf[:])
                stbh.append(sb)

            # Small inputs via the (otherwise idle) GpSimd queue
            sc_t = bigpool.tile([128, 2 * NB], f32, tag="sc")
            nc.gpsimd.dma_start(sc_t[:], sc[:])
            gwb_t = bigpool.tile([128, 2 * M], f32, tag="gwb")
            nc.gpsimd.dma_start(gwb_t[:], gwb[:])
            pb_t = bigpool.tile([128, O], f32, tag="pb")
            nc.gpsimd.dma_start(pb_t[:], pb[:])

            # Gating, batched per activation function to minimize ACT
            # table loads: all Exp together, all Sigmoid together.
            logits, nmxs, es, sms, rins, mixs, ctiles = [], [], [], [], [], [], []
            for j in range(NB):
                s = sc_t[:, j:j + 1]
                logit = gpool.tile([128, M], f32, tag="logit")
                nc.vector.scalar_tensor_tensor(
                    logit[:], gwb_t[:, 0:M], s, gwb_t[:, M:2 * M],
                    Alu.mult, Alu.add)
                logits.append(logit)
                nmx = gpool.tile([128, 1], f32, tag="nmx")
                nc.vector.tensor_reduce(
                    nmx[:], logit[:], axis=AxX, op=Alu.max, negate=True)
                nmxs.append(nmx)
            for j in range(NB):
                e = gpool.tile([128, M], f32, tag="e")
                nc.scalar.activation(e[:], logits[j][:], Act.Exp, bias=nmxs[j][:])
                es.append(e)
            for j in range(NB):
                mix = gpool.tile([128, 1], f32, tag="mix")
                nc.scalar.activation(
                    mix[:], sc_t[:, NB + j:NB + j + 1], Act.Sigmoid)
                mixs.append(mix)
            for j in range(NB):
                sm = gpool.tile([128, 1], f32, tag="sm")
                nc.vector.reduce_sum(sm[:], es[j][:], axis=AxX)
                rin = gpool.tile([128, 1], f32, tag="rin")
                nc.vector.reciprocal(rin[:], sm[:])
                c = cpool.tile([128, M + 1], f32, tag="c")
                nc.vector.tensor_scalar(
                    c[:, 0:M], es[j][:], rin[:], mixs[j][:], Alu.mult, Alu.mult)
                nc.vector.tensor_scalar(
                    c[:, M:M + 1], mixs[j][:], -1.0, 1.0, Alu.mult, Alu.add)
                ctiles.append(c)

            # acc_b starts as (1-mix) * prj_b
            atiles = []
            for j in range(NB):
                a = apool.tile([128, O], f32, tag="acc")
                nc.vector.tensor_scalar(
                    a[:], pb_t[:], ctiles[j][:, M:M + 1], None, Alu.mult)
                atiles.append(a)

            for o in range(NO):
                osl = slice(o * 512, (o + 1) * 512)
                for k in range(M + 1):
                    wchunk_next = (
                        load_w_chunk(o, k + 1) if k < M
                        else (load_w_chunk(o + 1, 0) if o < NO - 1 else None))
                    for b in range(NB):
                        ps = ppool.tile([128, 512], f32, tag="ps")
                        for h in range(NH):
                            nc.tensor.matmul(
                                ps[:],
                                lhsT=stbh[h][:, b * 128:(b + 1) * 128],
                                rhs=wchunk[h][:],
                                start=(h == 0),
                                stop=(h == NH - 1),
                            )
                        nc.vector.scalar_tensor_tensor(
                            atiles[b][:, osl], ps[:], ctiles[b][:, k:k + 1],
                            atiles[b][:, osl], Alu.mult, Alu.add)
                        if k == M:
                            # this o-half of acc[b] is final: drain it now
                            nc.sync.dma_start(
                                out_r[:, b, osl], atiles[b][:, osl])
                    wchunk = wchunk_next

    nc.compile()
    return nc


def get_nc():
    global _cached_nc
    if _cached_nc is None:
        _cached_nc = _build_nc()
    return _cached_nc


def make_in_maps(state, spectral_entropy, curvature, modulation_basis,
                 gate_w, gate_b, prj_w, prj_b):
    gwb = np.zeros((128, 2 * M), np.float32)
    gwb[:, 0:M] = np.asarray(gate_w, np.float32).reshape(1, M)
    gwb[:, M:2 * M] = np.asarray(gate_b, np.float32).reshape(1, M)
    pb = np.ascontiguousarray(
        np.broadcast_to(np.asarray(prj_b, np.float32).reshape(1, O), (128, O)))
    basis_c = np.ascontiguousarray(modulation_basis, dtype=np.float32)
    prj_c = np.ascontiguousarray(prj_w, dtype=np.float32)
    in_maps = []
    for c in range(NCORES):
        sl = slice(c * BL, (c + 1) * BL)
        stT = np.ascontiguousarray(np.asarray(state[sl], np.float32).T)
        sc = np.empty((128, 2 * NB), np.float32)
        sc[:, 0:NB] = np.asarray(
            spectral_entropy[sl], np.float32).reshape(NB, 128).T
        sc[:, NB:2 * NB] = np.asarray(
            curvature[sl], np.float32).reshape(NB, 128).T
        in_maps.append({"stateT": stT, "sc": sc, "basis": basis_c,
                        "prj_w": prj_c, "gwb": gwb, "pb": pb})
    return in_maps


def _install_ntff_hook():
    """Register the axon NTFF profiling hook if the image's antenv lacks it."""
    import sys, types
    if 'antenv.axon_hooks' in sys.modules:
        return
    mod = types.ModuleType('antenv.axon_hooks')
    mod._hook = None
    mod.set_axon_ntff_profile_hook = lambda h: setattr(mod, '_hook', h)
    mod.get_axon_ntff_profile_hook = lambda: mod._hook
    sys.modules['antenv.axon_hooks'] = mod
    import antenv
    antenv.axon_hooks = mod
    try:
        from trn_agent_boot.trn_boot import _ntff_profile_via_ctypes
        mod._hook = _ntff_profile_via_ctypes('/opt/axon/libaxon_pjrt.so')
    except Exception:
        pass


def kernel(state, spectral_entropy, curvature, modulation_basis,
           gate_w, gate_b, prj_w, prj_b):
    global LAST_EXEC_TIME_NS, LAST_TRACE
    from concourse import bass_utils

    nc = get_nc()
    in_maps = make_in_maps(state, spectral_entropy, curvature,
                           modulation_basis, gate_w, gate_b, prj_w, prj_b)

    trace = bool(int(os.environ.get("KERNEL_TRACE", "0")))
    kwargs = {}
    if trace:
        _install_ntff_hook()
        kwargs["trace"] = True

    res = bass_utils.run_bass_kernel_spmd(
        nc, in_maps, core_ids=list(range(NCORES)), **kwargs)
    LAST_EXEC_TIME_NS = res.exec_time_ns
    it = res.instructions_and_trace
    LAST_TRACE = it[1] if it else None
    return np.concatenate(
        [res.results[c]["out"] for c in range(NCORES)], axis=0)
